# revision 13
# baseline (speedup 1.0000x reference)
"""Cross-attention kernel for Trainium2, data-parallel over batch on 8 NeuronCores.

Per core (local batch BL=2):
  X staged bf16; X^T built by XBAR DMA transposes (no PE transposes).
  qT[a,lq] = Wq^T @ Xq^T      (fp16 weights x bf16 moving, f32 PSUM)
  kT[a,lk] = Wk^T @ Xkv^T     (fp16 x bf16)
  v[lk,a]  = Xkv^T.T @ Wv     (bf16 x fp16)
  sT[lk,lq]: lhsT=kT, rhs=qT  (fp16 x fp16, 11-bit significands)
  eT = exp(sT - G)            (ScalarE ACT, bf16 out; G fixed stabilizer)
  D[q] = sum_lk eT            (Vector tree-add + bf16 matmul rider)
  CT[a,lq] = v.T @ eT         (bf16)
  out[lq,e] = (CT.T @ Wo) * (1/D) + (bv @ Wo + bo)

Queues: sync hwdge = weights + staged input chunks; scalar hwdge = XBAR
transposes + output stores; gpsimd sw-dma = only the slow bo broadcast.
Both batches' staging is emitted before batch-0 compute so the second
batch's input pipeline never queues behind output stores.
"""
import numpy as np

import concourse.bass as bass
import concourse.bacc as bacc
import concourse.tile as tile
from concourse import mybir
from concourse.bass_utils import run_bass_kernel_spmd

B, LQ, LK, E, F, A = 16, 1024, 2048, 512, 256, 512
NCORES = 8
BL = B // NCORES
G = 100.0

f32 = mybir.dt.float32
f16 = mybir.dt.float16
bf16 = mybir.dt.bfloat16

QT = LQ // 128   # 8
KT = LK // 128   # 16
ET = E // 128    # 4
FT = F // 128    # 2
AT = A // 128    # 4
QC = LQ // 512   # 2  (also the number of Xq transpose groups)
KC = LK // 512   # 4  (also the number of Xk transpose groups)


def _body(tc):
    nc = tc.nc
    lstm = nc.dram_tensor("lstm_embeddings", [BL, LQ, E], bf16, kind="ExternalInput").ap()
    flow = nc.dram_tensor("optical_flow_features", [BL, LK, F], bf16, kind="ExternalInput").ap()
    Wq_d = nc.dram_tensor("Wq", [E, A], f16, kind="ExternalInput").ap()
    bq_d = nc.dram_tensor("bq", [A], f32, kind="ExternalInput").ap()
    Wk_d = nc.dram_tensor("Wk", [F, A], f16, kind="ExternalInput").ap()
    bk_d = nc.dram_tensor("bk", [A], f32, kind="ExternalInput").ap()
    Wv_d = nc.dram_tensor("Wv", [F, A], f16, kind="ExternalInput").ap()
    bv_d = nc.dram_tensor("bv", [A], f32, kind="ExternalInput").ap()
    Wo_d = nc.dram_tensor("Wo", [A, E], bf16, kind="ExternalInput").ap()
    bo_d = nc.dram_tensor("bo", [E], f32, kind="ExternalInput").ap()
    out_d = nc.dram_tensor("out", [BL, LQ, E], f32, kind="ExternalOutput").ap()

    from contextlib import ExitStack
    with ExitStack() as ctx:
        wp = ctx.enter_context(tc.tile_pool(name="wp", bufs=1))
        stage = ctx.enter_context(tc.tile_pool(name="stage", bufs=1))
        big = ctx.enter_context(tc.tile_pool(name="big", bufs=1))
        small = ctx.enter_context(tc.tile_pool(name="small", bufs=2))
        pp = ctx.enter_context(tc.tile_pool(name="pp", bufs=7, space="PSUM"))
        pdp = ctx.enter_context(tc.tile_pool(name="pdp", bufs=1, space="PSUM"))

        # ---- persistent weights (sync queue, interleaved with first stages) ----
        Wq_h = wp.tile([128, ET, A], f16)
        Wk_h = wp.tile([128, FT, A], f16)
        Wv_h = wp.tile([128, FT, A], f16)
        Wo_bf = wp.tile([128, AT, E], bf16)
        bq_sb = wp.tile([128, AT], f32)
        bk_sb = wp.tile([128, AT], f32)
        bv_sb = wp.tile([128, AT], f32)
        boB = wp.tile([128, E], f32)

        negG = wp.tile([128, 1], f32)
        nc.vector.memset(negG[:], -G)
        ones128_bf = wp.tile([128, 128], bf16)
        nc.vector.memset(ones128_bf[:], 1.0)
        # dummy exp so the activation-table load hoists into the preamble,
        # before any DMA it could pick up a false dependency on
        warm = wp.tile([128, 1], f32)
        nc.scalar.activation(out=warm[:], in_=negG[:],
                             func=mybir.ActivationFunctionType.Exp)


        # ---- staging phase: the sync queue is a pure staging pipeline --
        # each XBAR transpose directly follows its chunk load in-queue, so
        # every dependency is satisfied by queue order with zero cross-queue
        # semaphore stalls. Weights go on scalar (emitted at top), which
        # otherwise only runs compute ops and output stores.
        def stage_phase(b, first):
            xq_st = stage.tile([128, QT, E], bf16, tag="stageq")
            xk_st = stage.tile([128, KT, F], bf16, tag="stagek")
            lstm_r = lstm[b].rearrange("(t p) e -> p t e", p=128)
            flow_r = flow[b].rearrange("(t p) f -> p t f", p=128)
            XqT = big.tile([128, QC, 4, ET, 128], bf16, name=f"xqt{b}", tag=f"xqt{b}")
            XkT = big.tile([128, KC, 4, FT, 128], bf16, name=f"xkt{b}", tag=f"xkt{b}")
            if first:
                # ~10 hw semaphore slots are shared by all DMA instructions and
                # recycled in emission order with a wait-for-previous-user
                # guard, so emission order here tracks need order closely:
                # loads stream on sync, transposes+weights on scalar.
                nc.sync.dma_start(xq_st[:, 0:4, :], lstm_r[:, 0:4, :])
                nc.scalar.dma_start(Wq_h[:], Wq_d.rearrange("(t p) a -> p t a", p=128))
                nc.sync.dma_start(xq_st[:, 4:8, :], lstm_r[:, 4:8, :])
                nc.scalar.dma_start(bq_sb[:], bq_d.rearrange("(t p) -> p t", p=128))
                nc.scalar.dma_start_transpose(XqT[:, 0], xq_st[:, 0:4, :])
                nc.sync.dma_start(xk_st[:, 0:4, :], flow_r[:, 0:4, :])
                nc.scalar.dma_start_transpose(XqT[:, 1], xq_st[:, 4:8, :])
                nc.scalar.dma_start(bk_sb[:], bk_d.rearrange("(t p) -> p t", p=128))
                nc.sync.dma_start(xk_st[:, 4:8, :], flow_r[:, 4:8, :])
                nc.scalar.dma_start(Wk_h[:], Wk_d.rearrange("(t p) a -> p t a", p=128))
                nc.scalar.dma_start_transpose(XkT[:, 0], xk_st[:, 0:4, :])
                nc.sync.dma_start(xk_st[:, 8:12, :], flow_r[:, 8:12, :])
                nc.scalar.dma_start_transpose(XkT[:, 1], xk_st[:, 4:8, :])
                nc.sync.dma_start(xk_st[:, 12:16, :], flow_r[:, 12:16, :])
                nc.scalar.dma_start(Wv_h[:], Wv_d.rearrange("(t p) a -> p t a", p=128))
                nc.scalar.dma_start_transpose(XkT[:, 2], xk_st[:, 8:12, :])
                nc.scalar.dma_start_transpose(XkT[:, 3], xk_st[:, 12:16, :])
                # late-need loads: emitted last so their recycled semaphore
                # slots guard on early-completing DMAs
                nc.scalar.dma_start(bv_sb[:], bv_d.rearrange("(t p) -> p t", p=128))
                nc.scalar.dma_start(Wo_bf[:], Wo_d.rearrange("(t p) e -> p t e", p=128))
                bo_bcast_ap = bass.AP(tensor=bo_d.tensor, offset=bo_d.offset,
                                      ap=[[0, 128]] + list(bo_d.ap))
                nc.gpsimd.dma_start(boB[:], bo_bcast_ap)
            else:
                # second batch: not latency-critical; keep everything on sync
                # so nothing queues in front of batch-0 compute on scalar
                nc.sync.dma_start(xq_st[:], lstm_r[:])
                nc.sync.dma_start_transpose(XqT[:], xq_st[:])
                nc.sync.dma_start(xk_st[:, 0:8, :], flow_r[:, 0:8, :])
                nc.sync.dma_start_transpose(XkT[:, 0:2], xk_st[:, 0:8, :])
                nc.sync.dma_start(xk_st[:, 8:16, :], flow_r[:, 8:16, :])
                nc.sync.dma_start_transpose(XkT[:, 2:4], xk_st[:, 8:16, :])
            return XqT, XkT

        def compute_phase(b, XqT, XkT, bias_out, compute_bias):
            # B+C) projections and scores emitted in need order, so the PE
            # always has runnable work while the staging DMAs trickle in:
            # q(qc0), k(0), scores(qc0,kc0), q(qc1), k(1), scores(...), ...
            qT_c = [big.tile([128, AT, 512], f16, name=f"qTc{qc}", tag=f"qt{qc}") for qc in range(QC)]
            kT_c = [big.tile([128, AT, 512], f16, name=f"kTc{kc}", tag=f"kt{kc}") for kc in range(KC)]
            v_bf = big.tile([128, KT, A], bf16, tag="v")
            expT_c = [big.tile([128, KT, 512], bf16, name=f"expTc{qc}", tag=f"expt{qc}") for qc in range(QC)]
            dacc_c = [big.tile([128, 512], f32, name=f"daccc{qc}", tag=f"dacc{qc}") for qc in range(QC)]

            def q_proj(qc):
                for at in range(AT):
                    p = pp.tile([128, 512], f32, tag="pp")
                    for es in range(ET):
                        nc.tensor.matmul(
                            p[:], Wq_h[:, es, at * 128:(at + 1) * 128],
                            XqT[:, qc, :, es, :],
                            start=(es == 0), stop=(es == ET - 1))
                    nc.vector.tensor_scalar(
                        out=qT_c[qc][:, at, :], in0=p[:],
                        scalar1=bq_sb[:, at:at + 1], scalar2=None,
                        op0=mybir.AluOpType.add)

            def k_proj(kc):
                for at in range(AT):
                    p = pp.tile([128, 512], f32, tag="pp")
                    for fs in range(FT):
                        nc.tensor.matmul(
                            p[:], Wk_h[:, fs, at * 128:(at + 1) * 128],
                            XkT[:, kc, :, fs, :],
                            start=(fs == 0), stop=(fs == FT - 1))
                    nc.vector.tensor_scalar(
                        out=kT_c[kc][:, at, :], in0=p[:],
                        scalar1=bk_sb[:, at:at + 1], scalar2=None,
                        op0=mybir.AluOpType.add)

            def v_proj(lts):
                for lt in lts:
                    p = pp.tile([128, 512], f32, tag="pp")
                    for fs in range(FT):
                        nc.tensor.matmul(
                            p[:], XkT[:, lt // 4, lt % 4, fs, :], Wv_h[:, fs, :],
                            start=(fs == 0), stop=(fs == FT - 1))
                    nc.scalar.copy(v_bf[:, lt, :], p[:])

            def scores(qc, lts):
                for lt in lts:
                    kc, ko = lt // 4, lt % 4
                    p = pp.tile([128, 512], f32, tag="pp")
                    for at in range(AT):
                        nc.tensor.matmul(
                            p[:], kT_c[kc][:, at, ko * 128:(ko + 1) * 128],
                            qT_c[qc][:, at, :],
                            start=(at == 0), stop=(at == AT - 1))
                    nc.scalar.activation(
                        out=expT_c[qc][:, lt, :], in_=p[:],
                        func=mybir.ActivationFunctionType.Exp,
                        bias=negG[:], scale=1.0)
                    if lt == 0:
                        nc.vector.tensor_copy(dacc_c[qc][:], expT_c[qc][:, 0, :])
                    else:
                        nc.vector.tensor_add(dacc_c[qc][:], dacc_c[qc][:],
                                             expT_c[qc][:, lt, :])

            q_proj(0)
            k_proj(0)
            scores(0, range(0, 4))
            q_proj(1)
            k_proj(1)
            scores(0, range(4, 8))
            scores(1, range(0, 4))
            k_proj(2)
            v_proj(range(0, 8))
            scores(0, range(8, 12))
            scores(1, range(4, 8))
            k_proj(3)
            v_proj(range(8, 16))
            scores(0, range(12, 16))
            if compute_bias:
                # bias_out[p,e] = sum_a bv[a]*Wo[a,e] + bo[e]; emitted
                # mid-stream so a late Wo load can never stall the PE head
                ps_bo = pp.tile([128, E], f32, tag="pp")
                for at in range(AT):
                    bv_rep = small.tile([128, 128], bf16, tag="bvrep")
                    nc.vector.tensor_scalar_mul(bv_rep[:], ones128_bf[:],
                                                bv_sb[:, at:at + 1])
                    nc.tensor.matmul(ps_bo[:], bv_rep[:], Wo_bf[:, at, :],
                                     start=(at == 0), stop=(at == AT - 1))
                nc.vector.tensor_add(bias_out[:], ps_bo[:], boB[:])
            scores(1, range(8, 16))

            ps_d = pdp.tile([128, 8], f32, tag="pd")
            recipD = small.tile([128, 8], f32, tag="recip")

            # D) context (unnormalized, transposed) + final projection
            CT_c = [big.tile([128, AT, 512], bf16, name=f"CTc{qc}", tag=f"ct{qc}") for qc in range(QC)]
            for qc in range(QC):
                for at in range(AT):
                    p = pp.tile([128, 512], f32, tag="pp")
                    for lt in range(KT):
                        nc.tensor.matmul(
                            p[:], v_bf[:, lt, at * 128:(at + 1) * 128],
                            expT_c[qc][:, lt, :],
                            start=(lt == 0), stop=(lt == KT - 1))
                    nc.scalar.copy(CT_c[qc][:, at, :], p[:])

                dacc_bf = small.tile([128, 512], bf16, name=f"daccbf{qc}",
                                     tag=f"daccbf{qc}")
                nc.vector.tensor_copy(dacc_bf[:], dacc_c[qc][:])
                for qo in range(4):
                    qt = qc * 4 + qo
                    nc.tensor.matmul(ps_d[:, qt:qt + 1],
                                     dacc_bf[:, qo * 128:(qo + 1) * 128],
                                     ones128_bf[:, 0:1],
                                     start=True, stop=True)
                nc.vector.reciprocal(recipD[:, qc * 4:(qc + 1) * 4],
                                     ps_d[:, qc * 4:(qc + 1) * 4])

                for qo in range(4):
                    qt = qc * 4 + qo
                    p = pp.tile([128, 512], f32, tag="pp")
                    for at in range(AT):
                        nc.tensor.matmul(
                            p[:], CT_c[qc][:, at, qo * 128:(qo + 1) * 128],
                            Wo_bf[:, at, :],
                            start=(at == 0), stop=(at == AT - 1))
                    o_sb = small.tile([128, E], f32, tag="osb")
                    nc.vector.tensor_scalar(
                        out=o_sb[:], in0=p[:], scalar1=recipD[:, qt:qt + 1],
                        scalar2=None, op0=mybir.AluOpType.mult)
                    nc.vector.tensor_add(o_sb[:], o_sb[:], bias_out[:])
                    nc.scalar.dma_start(out_d[b, qt * 128:(qt + 1) * 128, :], o_sb[:])

        bias_out = wp.tile([128, E], f32)
        staged = [stage_phase(b, first=(b == 0)) for b in range(BL)]
        for b in range(BL):
            compute_phase(b, *staged[b], bias_out, compute_bias=(b == 0))


_NC_CACHE = []


def _get_nc():
    if not _NC_CACHE:
        nc = bacc.Bacc("TRN2", target_bir_lowering=False, debug=False)
        with tile.TileContext(nc) as tc:
            _body(tc)
        nc.compile()
        _NC_CACHE.append(nc)
    return _NC_CACHE[0]


def kernel(trace=False, **inputs):
    import ml_dtypes
    bf = ml_dtypes.bfloat16
    lstm = np.ascontiguousarray(
        np.asarray(inputs["lstm_embeddings"], dtype=np.float32).astype(bf))
    flow = np.ascontiguousarray(
        np.asarray(inputs["optical_flow_features"], dtype=np.float32).astype(bf))
    base = {k: np.ascontiguousarray(np.asarray(inputs[k], dtype=np.float32))
            for k in ("bq", "bk", "bv", "bo")}
    for k in ("Wq", "Wk", "Wv"):
        base[k] = np.ascontiguousarray(
            np.asarray(inputs[k], dtype=np.float32).astype(np.float16))
    base["Wo"] = np.ascontiguousarray(
        np.asarray(inputs["Wo"], dtype=np.float32).astype(bf))

    nc = _get_nc()
    in_maps = []
    for c in range(NCORES):
        m = dict(base)
        m["lstm_embeddings"] = lstm[c * BL:(c + 1) * BL]
        m["optical_flow_features"] = flow[c * BL:(c + 1) * BL]
        in_maps.append(m)

    kw = {}
    if trace:
        kw = dict(trace=True, trace_cores=[0])
    res = run_bass_kernel_spmd(nc, in_maps, core_ids=list(range(NCORES)), **kw)
    out = np.concatenate([r["out"] for r in res.results], axis=0)
    if trace:
        return out, res
    return out


# revision 14
# speedup vs baseline: 1.0928x; 1.0928x over previous
"""Cross-attention kernel for Trainium2, data-parallel over batch on 8 NeuronCores.

Per core (local batch BL=2):
  X staged bf16; X^T built by XBAR DMA transposes (no PE transposes).
  qT[a,lq] = Wq^T @ Xq^T      (fp16 weights x bf16 moving, f32 PSUM)
  kT[a,lk] = Wk^T @ Xkv^T     (fp16 x bf16)
  v[lk,a]  = Xkv^T.T @ Wv     (bf16 x fp16)
  sT[lk,lq]: lhsT=kT, rhs=qT  (fp16 x fp16, 11-bit significands)
  eT = exp(sT - G)            (ScalarE ACT, bf16 out; G fixed stabilizer)
  D[q] = sum_lk eT            (Vector tree-add + bf16 matmul rider)
  CT[a,lq] = v.T @ eT         (bf16)
  out[lq,e] = (CT.T @ Wo) * (1/D) + (bv @ Wo + bo)

Queues: sync hwdge = weights + staged input chunks; scalar hwdge = XBAR
transposes + output stores; gpsimd sw-dma = only the slow bo broadcast.
Both batches' staging is emitted before batch-0 compute so the second
batch's input pipeline never queues behind output stores.
"""
import numpy as np

import concourse.bass as bass
import concourse.bacc as bacc
import concourse.tile as tile
from concourse import mybir
from concourse.bass_utils import run_bass_kernel_spmd

B, LQ, LK, E, F, A = 16, 1024, 2048, 512, 256, 512
NCORES = 8
BL = B // NCORES
G = 100.0

f32 = mybir.dt.float32
f16 = mybir.dt.float16
bf16 = mybir.dt.bfloat16

QT = LQ // 128   # 8
KT = LK // 128   # 16
ET = E // 128    # 4
FT = F // 128    # 2
AT = A // 128    # 4
QC = LQ // 512   # 2  (also the number of Xq transpose groups)
KC = LK // 512   # 4  (also the number of Xk transpose groups)


def _body(tc):
    nc = tc.nc
    lstm = nc.dram_tensor("lstm_embeddings", [BL, LQ, E], bf16, kind="ExternalInput").ap()
    flow = nc.dram_tensor("optical_flow_features", [BL, LK, F], bf16, kind="ExternalInput").ap()
    Wq_d = nc.dram_tensor("Wq", [E, A], f16, kind="ExternalInput").ap()
    bq_d = nc.dram_tensor("bq", [A], f32, kind="ExternalInput").ap()
    Wk_d = nc.dram_tensor("Wk", [F, A], f16, kind="ExternalInput").ap()
    bk_d = nc.dram_tensor("bk", [A], f32, kind="ExternalInput").ap()
    Wv_d = nc.dram_tensor("Wv", [F, A], f16, kind="ExternalInput").ap()
    bv_d = nc.dram_tensor("bv", [A], f32, kind="ExternalInput").ap()
    Wo_d = nc.dram_tensor("Wo", [A, E], bf16, kind="ExternalInput").ap()
    bo_d = nc.dram_tensor("bo", [E], f32, kind="ExternalInput").ap()
    out_d = nc.dram_tensor("out", [BL, LQ, E], f32, kind="ExternalOutput").ap()

    from contextlib import ExitStack
    with ExitStack() as ctx:
        wp = ctx.enter_context(tc.tile_pool(name="wp", bufs=1))
        stage = ctx.enter_context(tc.tile_pool(name="stage", bufs=1))
        big = ctx.enter_context(tc.tile_pool(name="big", bufs=1))
        small = ctx.enter_context(tc.tile_pool(name="small", bufs=2))
        pp = ctx.enter_context(tc.tile_pool(name="pp", bufs=7, space="PSUM"))
        pdp = ctx.enter_context(tc.tile_pool(name="pdp", bufs=1, space="PSUM"))

        # ---- persistent weights (sync queue, interleaved with first stages) ----
        Wq_h = wp.tile([128, ET, A], f16)
        Wk_h = wp.tile([128, FT, A], f16)
        Wv_h = wp.tile([128, FT, A], f16)
        Wo_bf = wp.tile([128, AT, E], bf16)
        bq_sb = wp.tile([128, AT], f32)
        bk_sb = wp.tile([128, AT], f32)
        bv_sb = wp.tile([128, AT], f32)
        boB = wp.tile([128, E], f32)

        negG = wp.tile([128, 1], f32)
        nc.vector.memset(negG[:], -G)
        ones128_bf = wp.tile([128, 128], bf16)
        nc.vector.memset(ones128_bf[:], 1.0)
        # dummy exp so the activation-table load hoists into the preamble,
        # before any DMA it could pick up a false dependency on
        warm = wp.tile([128, 1], f32)
        nc.scalar.activation(out=warm[:], in_=negG[:],
                             func=mybir.ActivationFunctionType.Exp)


        # ---- staging phase: the sync queue is a pure staging pipeline --
        # each XBAR transpose directly follows its chunk load in-queue, so
        # every dependency is satisfied by queue order with zero cross-queue
        # semaphore stalls. Weights go on scalar (emitted at top), which
        # otherwise only runs compute ops and output stores.
        def stage_phase(b, first):
            xq_st = stage.tile([128, QT, E], bf16, tag="stageq")
            xk_st = stage.tile([128, KT, F], bf16, tag="stagek")
            lstm_r = lstm[b].rearrange("(t p) e -> p t e", p=128)
            flow_r = flow[b].rearrange("(t p) f -> p t f", p=128)
            XqT = big.tile([128, QC, 4, ET, 128], bf16, name=f"xqt{b}", tag=f"xqt{b}")
            XkT = big.tile([128, KC, 4, FT, 128], bf16, name=f"xkt{b}", tag=f"xkt{b}")
            if first:
                # single in-order staging chain on sync (load then its XBAR
                # transpose), k-side first; weights stream on scalar in the
                # order the PE consumes them
                nc.scalar.dma_start(bk_sb[:], bk_d.rearrange("(t p) -> p t", p=128))
                nc.scalar.dma_start(Wk_h[:], Wk_d.rearrange("(t p) a -> p t a", p=128))
                nc.sync.dma_start(xk_st[:, 0:4, :], flow_r[:, 0:4, :])
                nc.sync.dma_start_transpose(XkT[:, 0], xk_st[:, 0:4, :])
                nc.scalar.dma_start(bq_sb[:], bq_d.rearrange("(t p) -> p t", p=128))
                nc.scalar.dma_start(Wq_h[:], Wq_d.rearrange("(t p) a -> p t a", p=128))
                nc.sync.dma_start(xq_st[:, 0:4, :], lstm_r[:, 0:4, :])
                nc.sync.dma_start_transpose(XqT[:, 0], xq_st[:, 0:4, :])
                nc.sync.dma_start(xk_st[:, 4:8, :], flow_r[:, 4:8, :])
                nc.sync.dma_start_transpose(XkT[:, 1], xk_st[:, 4:8, :])
                nc.sync.dma_start(xq_st[:, 4:8, :], lstm_r[:, 4:8, :])
                nc.sync.dma_start_transpose(XqT[:, 1], xq_st[:, 4:8, :])
                nc.scalar.dma_start(bv_sb[:], bv_d.rearrange("(t p) -> p t", p=128))
                nc.scalar.dma_start(Wv_h[:], Wv_d.rearrange("(t p) a -> p t a", p=128))
                nc.sync.dma_start(xk_st[:, 8:12, :], flow_r[:, 8:12, :])
                nc.sync.dma_start_transpose(XkT[:, 2], xk_st[:, 8:12, :])
                nc.sync.dma_start(xk_st[:, 12:16, :], flow_r[:, 12:16, :])
                nc.sync.dma_start_transpose(XkT[:, 3], xk_st[:, 12:16, :])
                nc.scalar.dma_start(Wo_bf[:], Wo_d.rearrange("(t p) e -> p t e", p=128))
            else:
                # second batch: not latency-critical; keep everything on sync
                # so nothing queues in front of batch-0 compute on scalar
                nc.sync.dma_start(xq_st[:], lstm_r[:])
                nc.sync.dma_start_transpose(XqT[:], xq_st[:])
                nc.sync.dma_start(xk_st[:, 0:8, :], flow_r[:, 0:8, :])
                nc.sync.dma_start_transpose(XkT[:, 0:2], xk_st[:, 0:8, :])
                nc.sync.dma_start(xk_st[:, 8:16, :], flow_r[:, 8:16, :])
                nc.sync.dma_start_transpose(XkT[:, 2:4], xk_st[:, 8:16, :])
                bo_bcast_ap = bass.AP(tensor=bo_d.tensor, offset=bo_d.offset,
                                      ap=[[0, 128]] + list(bo_d.ap))
                nc.gpsimd.dma_start(boB[:], bo_bcast_ap)
            return XqT, XkT

        def compute_phase(b, XqT, XkT, bias_out, compute_bias):
            # B+C) projections and scores emitted in need order, so the PE
            # always has runnable work while the staging DMAs trickle in:
            # q(qc0), k(0), scores(qc0,kc0), q(qc1), k(1), scores(...), ...
            qT_c = [big.tile([128, AT, 512], f16, name=f"qTc{qc}", tag=f"qt{qc}") for qc in range(QC)]
            kT_c = [big.tile([128, AT, 512], f16, name=f"kTc{kc}", tag=f"kt{kc}") for kc in range(KC)]
            v_bf = big.tile([128, KT, A], bf16, tag="v")
            expT_c = [big.tile([128, KT, 512], bf16, name=f"expTc{qc}", tag=f"expt{qc}") for qc in range(QC)]
            dacc_c = [big.tile([128, 512], f32, name=f"daccc{qc}", tag=f"dacc{qc}") for qc in range(QC)]

            def q_proj(qc):
                for at in range(AT):
                    p = pp.tile([128, 512], f32, tag="pp")
                    for es in range(ET):
                        nc.tensor.matmul(
                            p[:], Wq_h[:, es, at * 128:(at + 1) * 128],
                            XqT[:, qc, :, es, :],
                            start=(es == 0), stop=(es == ET - 1))
                    nc.vector.tensor_scalar(
                        out=qT_c[qc][:, at, :], in0=p[:],
                        scalar1=bq_sb[:, at:at + 1], scalar2=None,
                        op0=mybir.AluOpType.add)

            def k_proj(kc):
                for at in range(AT):
                    p = pp.tile([128, 512], f32, tag="pp")
                    for fs in range(FT):
                        nc.tensor.matmul(
                            p[:], Wk_h[:, fs, at * 128:(at + 1) * 128],
                            XkT[:, kc, :, fs, :],
                            start=(fs == 0), stop=(fs == FT - 1))
                    nc.vector.tensor_scalar(
                        out=kT_c[kc][:, at, :], in0=p[:],
                        scalar1=bk_sb[:, at:at + 1], scalar2=None,
                        op0=mybir.AluOpType.add)

            def v_proj(lts):
                for lt in lts:
                    p = pp.tile([128, 512], f32, tag="pp")
                    for fs in range(FT):
                        nc.tensor.matmul(
                            p[:], XkT[:, lt // 4, lt % 4, fs, :], Wv_h[:, fs, :],
                            start=(fs == 0), stop=(fs == FT - 1))
                    nc.scalar.copy(v_bf[:, lt, :], p[:])

            def scores(qc, lts):
                for lt in lts:
                    kc, ko = lt // 4, lt % 4
                    p = pp.tile([128, 512], f32, tag="pp")
                    for at in range(AT):
                        nc.tensor.matmul(
                            p[:], kT_c[kc][:, at, ko * 128:(ko + 1) * 128],
                            qT_c[qc][:, at, :],
                            start=(at == 0), stop=(at == AT - 1))
                    nc.scalar.activation(
                        out=expT_c[qc][:, lt, :], in_=p[:],
                        func=mybir.ActivationFunctionType.Exp,
                        bias=negG[:], scale=1.0)
                    if lt == 0:
                        nc.vector.tensor_copy(dacc_c[qc][:], expT_c[qc][:, 0, :])
                    else:
                        nc.vector.tensor_add(dacc_c[qc][:], dacc_c[qc][:],
                                             expT_c[qc][:, lt, :])

            k_proj(0)
            q_proj(0)
            scores(0, range(0, 4))
            k_proj(1)
            q_proj(1)
            scores(0, range(4, 8))
            scores(1, range(0, 4))
            k_proj(2)
            v_proj(range(0, 8))
            scores(0, range(8, 12))
            scores(1, range(4, 8))
            k_proj(3)
            v_proj(range(8, 16))
            scores(0, range(12, 16))
            if compute_bias:
                # bias_out[p,e] = sum_a bv[a]*Wo[a,e] + bo[e]; emitted
                # mid-stream so a late Wo load can never stall the PE head
                ps_bo = pp.tile([128, E], f32, tag="pp")
                for at in range(AT):
                    bv_rep = small.tile([128, 128], bf16, tag="bvrep")
                    nc.vector.tensor_scalar_mul(bv_rep[:], ones128_bf[:],
                                                bv_sb[:, at:at + 1])
                    nc.tensor.matmul(ps_bo[:], bv_rep[:], Wo_bf[:, at, :],
                                     start=(at == 0), stop=(at == AT - 1))
                nc.vector.tensor_add(bias_out[:], ps_bo[:], boB[:])
            scores(1, range(8, 16))

            ps_d = pdp.tile([128, 8], f32, tag="pd")
            recipD = small.tile([128, 8], f32, tag="recip")

            # D) context (unnormalized, transposed) + final projection
            CT_c = [big.tile([128, AT, 512], bf16, name=f"CTc{qc}", tag=f"ct{qc}") for qc in range(QC)]
            for qc in range(QC):
                for at in range(AT):
                    p = pp.tile([128, 512], f32, tag="pp")
                    for lt in range(KT):
                        nc.tensor.matmul(
                            p[:], v_bf[:, lt, at * 128:(at + 1) * 128],
                            expT_c[qc][:, lt, :],
                            start=(lt == 0), stop=(lt == KT - 1))
                    nc.scalar.copy(CT_c[qc][:, at, :], p[:])

                dacc_bf = small.tile([128, 512], bf16, name=f"daccbf{qc}",
                                     tag=f"daccbf{qc}")
                nc.vector.tensor_copy(dacc_bf[:], dacc_c[qc][:])
                for qo in range(4):
                    qt = qc * 4 + qo
                    nc.tensor.matmul(ps_d[:, qt:qt + 1],
                                     dacc_bf[:, qo * 128:(qo + 1) * 128],
                                     ones128_bf[:, 0:1],
                                     start=True, stop=True)
                nc.vector.reciprocal(recipD[:, qc * 4:(qc + 1) * 4],
                                     ps_d[:, qc * 4:(qc + 1) * 4])

                for qo in range(4):
                    qt = qc * 4 + qo
                    p = pp.tile([128, 512], f32, tag="pp")
                    for at in range(AT):
                        nc.tensor.matmul(
                            p[:], CT_c[qc][:, at, qo * 128:(qo + 1) * 128],
                            Wo_bf[:, at, :],
                            start=(at == 0), stop=(at == AT - 1))
                    o_sb = small.tile([128, E], f32, tag="osb")
                    nc.vector.tensor_scalar(
                        out=o_sb[:], in0=p[:], scalar1=recipD[:, qt:qt + 1],
                        scalar2=None, op0=mybir.AluOpType.mult)
                    nc.vector.tensor_add(o_sb[:], o_sb[:], bias_out[:])
                    nc.scalar.dma_start(out_d[b, qt * 128:(qt + 1) * 128, :], o_sb[:])

        bias_out = wp.tile([128, E], f32)
        staged = [stage_phase(b, first=(b == 0)) for b in range(BL)]
        for b in range(BL):
            compute_phase(b, *staged[b], bias_out, compute_bias=(b == 0))


_NC_CACHE = []


def _get_nc():
    if not _NC_CACHE:
        nc = bacc.Bacc("TRN2", target_bir_lowering=False, debug=False)
        with tile.TileContext(nc) as tc:
            _body(tc)
        nc.compile()
        _NC_CACHE.append(nc)
    return _NC_CACHE[0]


def kernel(trace=False, **inputs):
    import ml_dtypes
    bf = ml_dtypes.bfloat16
    lstm = np.ascontiguousarray(
        np.asarray(inputs["lstm_embeddings"], dtype=np.float32).astype(bf))
    flow = np.ascontiguousarray(
        np.asarray(inputs["optical_flow_features"], dtype=np.float32).astype(bf))
    base = {k: np.ascontiguousarray(np.asarray(inputs[k], dtype=np.float32))
            for k in ("bq", "bk", "bv", "bo")}
    for k in ("Wq", "Wk", "Wv"):
        base[k] = np.ascontiguousarray(
            np.asarray(inputs[k], dtype=np.float32).astype(np.float16))
    base["Wo"] = np.ascontiguousarray(
        np.asarray(inputs["Wo"], dtype=np.float32).astype(bf))

    nc = _get_nc()
    in_maps = []
    for c in range(NCORES):
        m = dict(base)
        m["lstm_embeddings"] = lstm[c * BL:(c + 1) * BL]
        m["optical_flow_features"] = flow[c * BL:(c + 1) * BL]
        in_maps.append(m)

    kw = {}
    if trace:
        kw = dict(trace=True, trace_cores=[0])
    res = run_bass_kernel_spmd(nc, in_maps, core_ids=list(range(NCORES)), **kw)
    out = np.concatenate([r["out"] for r in res.results], axis=0)
    if trace:
        return out, res
    return out


# revision 15
# speedup vs baseline: 1.0966x; 1.0035x over previous
"""Cross-attention kernel for Trainium2, data-parallel over batch on 8 NeuronCores.

Per core (local batch BL=2):
  X staged bf16; X^T built by XBAR DMA transposes (no PE transposes).
  qT[a,lq] = Wq^T @ Xq^T      (fp16 weights x bf16 moving, f32 PSUM)
  kT[a,lk] = Wk^T @ Xkv^T     (fp16 x bf16)
  v[lk,a]  = Xkv^T.T @ Wv     (bf16 x fp16)
  sT[lk,lq]: lhsT=kT, rhs=qT  (fp16 x fp16, 11-bit significands)
  eT = exp(sT - G)            (ScalarE ACT, bf16 out; G fixed stabilizer)
  D[q] = sum_lk eT            (Vector tree-add + bf16 matmul rider)
  CT[a,lq] = v.T @ eT         (bf16)
  out[lq,e] = (CT.T @ Wo) * (1/D) + (bv @ Wo + bo)

Queues: sync hwdge = weights + staged input chunks; scalar hwdge = XBAR
transposes + output stores; gpsimd sw-dma = only the slow bo broadcast.
Both batches' staging is emitted before batch-0 compute so the second
batch's input pipeline never queues behind output stores.
"""
import numpy as np

import concourse.bass as bass
import concourse.bacc as bacc
import concourse.tile as tile
from concourse import mybir
from concourse.bass_utils import run_bass_kernel_spmd

B, LQ, LK, E, F, A = 16, 1024, 2048, 512, 256, 512
NCORES = 8
BL = B // NCORES
G = 100.0

f32 = mybir.dt.float32
f16 = mybir.dt.float16
bf16 = mybir.dt.bfloat16

QT = LQ // 128   # 8
KT = LK // 128   # 16
ET = E // 128    # 4
FT = F // 128    # 2
AT = A // 128    # 4
QC = LQ // 512   # 2  (also the number of Xq transpose groups)
KC = LK // 512   # 4  (also the number of Xk transpose groups)


def _body(tc):
    nc = tc.nc
    lstm = nc.dram_tensor("lstm_embeddings", [BL, LQ, E], bf16, kind="ExternalInput").ap()
    flow = nc.dram_tensor("optical_flow_features", [BL, LK, F], bf16, kind="ExternalInput").ap()
    Wq_d = nc.dram_tensor("Wq", [E, A], f16, kind="ExternalInput").ap()
    bq_d = nc.dram_tensor("bq", [A], f32, kind="ExternalInput").ap()
    Wk_d = nc.dram_tensor("Wk", [F, A], f16, kind="ExternalInput").ap()
    bk_d = nc.dram_tensor("bk", [A], f32, kind="ExternalInput").ap()
    Wv_d = nc.dram_tensor("Wv", [F, A], f16, kind="ExternalInput").ap()
    bv_d = nc.dram_tensor("bv", [A], f32, kind="ExternalInput").ap()
    Wo_d = nc.dram_tensor("Wo", [A, E], bf16, kind="ExternalInput").ap()
    bo_d = nc.dram_tensor("bo", [E], f32, kind="ExternalInput").ap()
    out_d = nc.dram_tensor("out", [BL, LQ, E], f32, kind="ExternalOutput").ap()

    from contextlib import ExitStack
    with ExitStack() as ctx:
        wp = ctx.enter_context(tc.tile_pool(name="wp", bufs=1))
        stage = ctx.enter_context(tc.tile_pool(name="stage", bufs=1))
        big = ctx.enter_context(tc.tile_pool(name="big", bufs=1))
        small = ctx.enter_context(tc.tile_pool(name="small", bufs=2))
        pp = ctx.enter_context(tc.tile_pool(name="pp", bufs=7, space="PSUM"))
        pdp = ctx.enter_context(tc.tile_pool(name="pdp", bufs=1, space="PSUM"))

        # ---- persistent weights (sync queue, interleaved with first stages) ----
        Wq_h = wp.tile([128, ET, A], f16)
        Wk_h = wp.tile([128, FT, A], f16)
        Wv_h = wp.tile([128, FT, A], f16)
        Wo_bf = wp.tile([128, AT, E], bf16)
        bq_sb = wp.tile([128, AT], f32)
        bk_sb = wp.tile([128, AT], f32)
        bv_sb = wp.tile([128, AT], f32)
        boB = wp.tile([128, E], f32)

        negG = wp.tile([128, 1], f32)
        nc.vector.memset(negG[:], -G)
        ones128_bf = wp.tile([128, 128], bf16)
        nc.vector.memset(ones128_bf[:], 1.0)
        # dummy exp so the activation-table load hoists into the preamble,
        # before any DMA it could pick up a false dependency on
        warm = wp.tile([128, 1], f32)
        nc.scalar.activation(out=warm[:], in_=negG[:],
                             func=mybir.ActivationFunctionType.Exp)


        # ---- staging phase: the sync queue is a pure staging pipeline --
        # each XBAR transpose directly follows its chunk load in-queue, so
        # every dependency is satisfied by queue order with zero cross-queue
        # semaphore stalls. Weights go on scalar (emitted at top), which
        # otherwise only runs compute ops and output stores.
        def stage_phase(b, first):
            xq_st = stage.tile([128, QT, E], bf16, tag="stageq")
            xk_st = stage.tile([128, KT, F], bf16, tag="stagek")
            lstm_r = lstm[b].rearrange("(t p) e -> p t e", p=128)
            flow_r = flow[b].rearrange("(t p) f -> p t f", p=128)
            XqT = big.tile([128, QC, 4, ET, 128], bf16, name=f"xqt{b}", tag=f"xqt{b}")
            XkT = big.tile([128, KC, 4, FT, 128], bf16, name=f"xkt{b}", tag=f"xkt{b}")
            if first:
                # single in-order staging chain on sync (load then its XBAR
                # transpose), k-side first; weights stream on scalar in the
                # order the PE consumes them
                nc.scalar.dma_start(bk_sb[:], bk_d.rearrange("(t p) -> p t", p=128))
                nc.scalar.dma_start(Wk_h[:], Wk_d.rearrange("(t p) a -> p t a", p=128))
                nc.sync.dma_start(xk_st[:, 0:4, :], flow_r[:, 0:4, :])
                nc.sync.dma_start(xq_st[:, 0:4, :], lstm_r[:, 0:4, :])
                nc.scalar.dma_start(bq_sb[:], bq_d.rearrange("(t p) -> p t", p=128))
                nc.scalar.dma_start(Wq_h[:], Wq_d.rearrange("(t p) a -> p t a", p=128))
                nc.sync.dma_start_transpose(XkT[:, 0], xk_st[:, 0:4, :])
                nc.sync.dma_start_transpose(XqT[:, 0], xq_st[:, 0:4, :])
                nc.sync.dma_start(xk_st[:, 4:8, :], flow_r[:, 4:8, :])
                nc.sync.dma_start(xq_st[:, 4:8, :], lstm_r[:, 4:8, :])
                nc.sync.dma_start_transpose(XkT[:, 1], xk_st[:, 4:8, :])
                nc.sync.dma_start_transpose(XqT[:, 1], xq_st[:, 4:8, :])
                nc.scalar.dma_start(bv_sb[:], bv_d.rearrange("(t p) -> p t", p=128))
                nc.scalar.dma_start(Wv_h[:], Wv_d.rearrange("(t p) a -> p t a", p=128))
                nc.sync.dma_start(xk_st[:, 8:12, :], flow_r[:, 8:12, :])
                nc.sync.dma_start_transpose(XkT[:, 2], xk_st[:, 8:12, :])
                nc.sync.dma_start(xk_st[:, 12:16, :], flow_r[:, 12:16, :])
                nc.sync.dma_start_transpose(XkT[:, 3], xk_st[:, 12:16, :])
                nc.scalar.dma_start(Wo_bf[:], Wo_d.rearrange("(t p) e -> p t e", p=128))
            else:
                # second batch: not latency-critical; keep everything on sync
                # so nothing queues in front of batch-0 compute on scalar
                nc.sync.dma_start(xq_st[:], lstm_r[:])
                nc.sync.dma_start_transpose(XqT[:], xq_st[:])
                nc.sync.dma_start(xk_st[:, 0:8, :], flow_r[:, 0:8, :])
                nc.sync.dma_start_transpose(XkT[:, 0:2], xk_st[:, 0:8, :])
                nc.sync.dma_start(xk_st[:, 8:16, :], flow_r[:, 8:16, :])
                nc.sync.dma_start_transpose(XkT[:, 2:4], xk_st[:, 8:16, :])
                bo_bcast_ap = bass.AP(tensor=bo_d.tensor, offset=bo_d.offset,
                                      ap=[[0, 128]] + list(bo_d.ap))
                nc.gpsimd.dma_start(boB[:], bo_bcast_ap)
            return XqT, XkT

        def compute_phase(b, XqT, XkT, bias_out, compute_bias):
            # B+C) projections and scores emitted in need order, so the PE
            # always has runnable work while the staging DMAs trickle in:
            # q(qc0), k(0), scores(qc0,kc0), q(qc1), k(1), scores(...), ...
            qT_c = [big.tile([128, AT, 512], f16, name=f"qTc{qc}", tag=f"qt{qc}") for qc in range(QC)]
            kT_c = [big.tile([128, AT, 512], f16, name=f"kTc{kc}", tag=f"kt{kc}") for kc in range(KC)]
            v_bf = big.tile([128, KT, A], bf16, tag="v")
            expT_c = [big.tile([128, KT, 512], bf16, name=f"expTc{qc}", tag=f"expt{qc}") for qc in range(QC)]
            dacc_c = [big.tile([128, 512], f32, name=f"daccc{qc}", tag=f"dacc{qc}") for qc in range(QC)]

            def q_proj(qc):
                for at in range(AT):
                    p = pp.tile([128, 512], f32, tag="pp")
                    for es in range(ET):
                        nc.tensor.matmul(
                            p[:], Wq_h[:, es, at * 128:(at + 1) * 128],
                            XqT[:, qc, :, es, :],
                            start=(es == 0), stop=(es == ET - 1))
                    nc.vector.tensor_scalar(
                        out=qT_c[qc][:, at, :], in0=p[:],
                        scalar1=bq_sb[:, at:at + 1], scalar2=None,
                        op0=mybir.AluOpType.add)

            def k_proj(kc):
                for at in range(AT):
                    p = pp.tile([128, 512], f32, tag="pp")
                    for fs in range(FT):
                        nc.tensor.matmul(
                            p[:], Wk_h[:, fs, at * 128:(at + 1) * 128],
                            XkT[:, kc, :, fs, :],
                            start=(fs == 0), stop=(fs == FT - 1))
                    nc.vector.tensor_scalar(
                        out=kT_c[kc][:, at, :], in0=p[:],
                        scalar1=bk_sb[:, at:at + 1], scalar2=None,
                        op0=mybir.AluOpType.add)

            def v_proj(lts):
                for lt in lts:
                    p = pp.tile([128, 512], f32, tag="pp")
                    for fs in range(FT):
                        nc.tensor.matmul(
                            p[:], XkT[:, lt // 4, lt % 4, fs, :], Wv_h[:, fs, :],
                            start=(fs == 0), stop=(fs == FT - 1))
                    nc.scalar.copy(v_bf[:, lt, :], p[:])

            def scores(qc, lts):
                for lt in lts:
                    kc, ko = lt // 4, lt % 4
                    p = pp.tile([128, 512], f32, tag="pp")
                    for at in range(AT):
                        nc.tensor.matmul(
                            p[:], kT_c[kc][:, at, ko * 128:(ko + 1) * 128],
                            qT_c[qc][:, at, :],
                            start=(at == 0), stop=(at == AT - 1))
                    nc.scalar.activation(
                        out=expT_c[qc][:, lt, :], in_=p[:],
                        func=mybir.ActivationFunctionType.Exp,
                        bias=negG[:], scale=1.0)
                    if lt == 0:
                        nc.vector.tensor_copy(dacc_c[qc][:], expT_c[qc][:, 0, :])
                    else:
                        nc.vector.tensor_add(dacc_c[qc][:], dacc_c[qc][:],
                                             expT_c[qc][:, lt, :])

            k_proj(0)
            q_proj(0)
            scores(0, range(0, 4))
            k_proj(1)
            q_proj(1)
            scores(0, range(4, 8))
            scores(1, range(0, 4))
            k_proj(2)
            v_proj(range(0, 8))
            scores(0, range(8, 12))
            scores(1, range(4, 8))
            k_proj(3)
            v_proj(range(8, 16))
            scores(0, range(12, 16))
            if compute_bias:
                # bias_out[p,e] = sum_a bv[a]*Wo[a,e] + bo[e]; emitted
                # mid-stream so a late Wo load can never stall the PE head
                ps_bo = pp.tile([128, E], f32, tag="pp")
                for at in range(AT):
                    bv_rep = small.tile([128, 128], bf16, tag="bvrep")
                    nc.vector.tensor_scalar_mul(bv_rep[:], ones128_bf[:],
                                                bv_sb[:, at:at + 1])
                    nc.tensor.matmul(ps_bo[:], bv_rep[:], Wo_bf[:, at, :],
                                     start=(at == 0), stop=(at == AT - 1))
                nc.vector.tensor_add(bias_out[:], ps_bo[:], boB[:])
            scores(1, range(8, 16))

            ps_d = pdp.tile([128, 8], f32, tag="pd")
            recipD = small.tile([128, 8], f32, tag="recip")

            # D) context (unnormalized, transposed) + final projection
            CT_c = [big.tile([128, AT, 512], bf16, name=f"CTc{qc}", tag=f"ct{qc}") for qc in range(QC)]
            for qc in range(QC):
                for at in range(AT):
                    p = pp.tile([128, 512], f32, tag="pp")
                    for lt in range(KT):
                        nc.tensor.matmul(
                            p[:], v_bf[:, lt, at * 128:(at + 1) * 128],
                            expT_c[qc][:, lt, :],
                            start=(lt == 0), stop=(lt == KT - 1))
                    nc.scalar.copy(CT_c[qc][:, at, :], p[:])

                dacc_bf = small.tile([128, 512], bf16, name=f"daccbf{qc}",
                                     tag=f"daccbf{qc}")
                nc.vector.tensor_copy(dacc_bf[:], dacc_c[qc][:])
                for qo in range(4):
                    qt = qc * 4 + qo
                    nc.tensor.matmul(ps_d[:, qt:qt + 1],
                                     dacc_bf[:, qo * 128:(qo + 1) * 128],
                                     ones128_bf[:, 0:1],
                                     start=True, stop=True)
                nc.vector.reciprocal(recipD[:, qc * 4:(qc + 1) * 4],
                                     ps_d[:, qc * 4:(qc + 1) * 4])

                for qo in range(4):
                    qt = qc * 4 + qo
                    p = pp.tile([128, 512], f32, tag="pp")
                    for at in range(AT):
                        nc.tensor.matmul(
                            p[:], CT_c[qc][:, at, qo * 128:(qo + 1) * 128],
                            Wo_bf[:, at, :],
                            start=(at == 0), stop=(at == AT - 1))
                    o_sb = small.tile([128, E], f32, tag="osb")
                    nc.vector.tensor_scalar(
                        out=o_sb[:], in0=p[:], scalar1=recipD[:, qt:qt + 1],
                        scalar2=None, op0=mybir.AluOpType.mult)
                    nc.vector.tensor_add(o_sb[:], o_sb[:], bias_out[:])
                    nc.scalar.dma_start(out_d[b, qt * 128:(qt + 1) * 128, :], o_sb[:])

        bias_out = wp.tile([128, E], f32)
        staged = [stage_phase(b, first=(b == 0)) for b in range(BL)]
        for b in range(BL):
            compute_phase(b, *staged[b], bias_out, compute_bias=(b == 0))


_NC_CACHE = []


def _get_nc():
    if not _NC_CACHE:
        nc = bacc.Bacc("TRN2", target_bir_lowering=False, debug=False)
        with tile.TileContext(nc) as tc:
            _body(tc)
        nc.compile()
        _NC_CACHE.append(nc)
    return _NC_CACHE[0]


def kernel(trace=False, **inputs):
    import ml_dtypes
    bf = ml_dtypes.bfloat16
    lstm = np.ascontiguousarray(
        np.asarray(inputs["lstm_embeddings"], dtype=np.float32).astype(bf))
    flow = np.ascontiguousarray(
        np.asarray(inputs["optical_flow_features"], dtype=np.float32).astype(bf))
    base = {k: np.ascontiguousarray(np.asarray(inputs[k], dtype=np.float32))
            for k in ("bq", "bk", "bv", "bo")}
    for k in ("Wq", "Wk", "Wv"):
        base[k] = np.ascontiguousarray(
            np.asarray(inputs[k], dtype=np.float32).astype(np.float16))
    base["Wo"] = np.ascontiguousarray(
        np.asarray(inputs["Wo"], dtype=np.float32).astype(bf))

    nc = _get_nc()
    in_maps = []
    for c in range(NCORES):
        m = dict(base)
        m["lstm_embeddings"] = lstm[c * BL:(c + 1) * BL]
        m["optical_flow_features"] = flow[c * BL:(c + 1) * BL]
        in_maps.append(m)

    kw = {}
    if trace:
        kw = dict(trace=True, trace_cores=[0])
    res = run_bass_kernel_spmd(nc, in_maps, core_ids=list(range(NCORES)), **kw)
    out = np.concatenate([r["out"] for r in res.results], axis=0)
    if trace:
        return out, res
    return out


# revision 16
# speedup vs baseline: 1.1106x; 1.0128x over previous
"""Cross-attention kernel for Trainium2, data-parallel over batch on 8 NeuronCores.

Per core (local batch BL=2):
  X staged bf16; X^T built by XBAR DMA transposes (no PE transposes).
  qT[a,lq] = Wq^T @ Xq^T      (fp16 weights x bf16 moving, f32 PSUM)
  kT[a,lk] = Wk^T @ Xkv^T     (fp16 x bf16)
  v[lk,a]  = Xkv^T.T @ Wv     (bf16 x fp16)
  sT[lk,lq]: lhsT=kT, rhs=qT  (fp16 x fp16, 11-bit significands)
  eT = exp(sT - G)            (ScalarE ACT, bf16 out; G fixed stabilizer)
  D[q] = sum_lk eT            (Vector tree-add + bf16 matmul rider)
  CT[a,lq] = v.T @ eT         (bf16)
  out[lq,e] = (CT.T @ Wo) * (1/D) + (bv @ Wo + bo)

Queues: sync hwdge = weights + staged input chunks; scalar hwdge = XBAR
transposes + output stores; gpsimd sw-dma = only the slow bo broadcast.
Both batches' staging is emitted before batch-0 compute so the second
batch's input pipeline never queues behind output stores.
"""
import numpy as np

import concourse.bass as bass
import concourse.bacc as bacc
import concourse.tile as tile
from concourse import mybir
from concourse.bass_utils import run_bass_kernel_spmd

B, LQ, LK, E, F, A = 16, 1024, 2048, 512, 256, 512
NCORES = 8
BL = B // NCORES
G = 100.0

f32 = mybir.dt.float32
f16 = mybir.dt.float16
bf16 = mybir.dt.bfloat16

QT = LQ // 128   # 8
KT = LK // 128   # 16
ET = E // 128    # 4
FT = F // 128    # 2
AT = A // 128    # 4
QC = LQ // 512   # 2  (also the number of Xq transpose groups)
KC = LK // 512   # 4  (also the number of Xk transpose groups)


def _body(tc):
    nc = tc.nc
    lstm = nc.dram_tensor("lstm_embeddings", [BL, LQ, E], bf16, kind="ExternalInput").ap()
    flow = nc.dram_tensor("optical_flow_features", [BL, LK, F], bf16, kind="ExternalInput").ap()
    Wq_d = nc.dram_tensor("Wq", [E, A], f16, kind="ExternalInput").ap()
    bq_d = nc.dram_tensor("bq", [A], f32, kind="ExternalInput").ap()
    Wk_d = nc.dram_tensor("Wk", [F, A], f16, kind="ExternalInput").ap()
    bk_d = nc.dram_tensor("bk", [A], f32, kind="ExternalInput").ap()
    Wv_d = nc.dram_tensor("Wv", [F, A], f16, kind="ExternalInput").ap()
    bv_d = nc.dram_tensor("bv", [A], f32, kind="ExternalInput").ap()
    Wo_d = nc.dram_tensor("Wo", [A, E], bf16, kind="ExternalInput").ap()
    bo_d = nc.dram_tensor("bo", [E], f32, kind="ExternalInput").ap()
    out_d = nc.dram_tensor("out", [BL, LQ, E], f32, kind="ExternalOutput").ap()

    from contextlib import ExitStack
    with ExitStack() as ctx:
        wp = ctx.enter_context(tc.tile_pool(name="wp", bufs=1))
        stage = ctx.enter_context(tc.tile_pool(name="stage", bufs=1))
        big = ctx.enter_context(tc.tile_pool(name="big", bufs=1))
        small = ctx.enter_context(tc.tile_pool(name="small", bufs=2))
        pp = ctx.enter_context(tc.tile_pool(name="pp", bufs=7, space="PSUM"))
        pdp = ctx.enter_context(tc.tile_pool(name="pdp", bufs=1, space="PSUM"))

        # ---- persistent weights (sync queue, interleaved with first stages) ----
        Wq_h = wp.tile([128, ET, A], f16)
        Wk_h = wp.tile([128, FT, A], f16)
        Wv_h = wp.tile([128, FT, A], f16)
        Wo_bf = wp.tile([128, AT, E], bf16)
        bq_sb = wp.tile([128, AT], f32)
        bk_sb = wp.tile([128, AT], f32)
        bv_sb = wp.tile([128, AT], f32)
        boB = wp.tile([128, E], f32)

        negG = wp.tile([128, 1], f32)
        nc.vector.memset(negG[:], -G)
        ones128_bf = wp.tile([128, 128], bf16)
        nc.vector.memset(ones128_bf[:], 1.0)
        # dummy exp so the activation-table load hoists into the preamble,
        # before any DMA it could pick up a false dependency on
        warm = wp.tile([128, 1], f32)
        nc.scalar.activation(out=warm[:], in_=negG[:],
                             func=mybir.ActivationFunctionType.Exp)


        # ---- staging phase: the sync queue is a pure staging pipeline --
        # each XBAR transpose directly follows its chunk load in-queue, so
        # every dependency is satisfied by queue order with zero cross-queue
        # semaphore stalls. Weights go on scalar (emitted at top), which
        # otherwise only runs compute ops and output stores.
        def stage_phase(b, first):
            xq_st = stage.tile([128, QT, E], bf16, tag="stageq")
            xk_st = stage.tile([128, KT, F], bf16, tag="stagek")
            lstm_r = lstm[b].rearrange("(t p) e -> p t e", p=128)
            flow_r = flow[b].rearrange("(t p) f -> p t f", p=128)
            XqT = big.tile([128, QC, 4, ET, 128], bf16, name=f"xqt{b}", tag=f"xqt{b}")
            XkT = big.tile([128, KC, 4, FT, 128], bf16, name=f"xkt{b}", tag=f"xkt{b}")
            if first:
                # single in-order staging chain on sync (load then its XBAR
                # transpose), k-side first; weights stream on scalar in the
                # order the PE consumes them
                nc.scalar.dma_start(bk_sb[:], bk_d.rearrange("(t p) -> p t", p=128))
                nc.scalar.dma_start(Wk_h[:], Wk_d.rearrange("(t p) a -> p t a", p=128))
                nc.sync.dma_start(xk_st[:, 0:4, :], flow_r[:, 0:4, :])
                nc.sync.dma_start(xq_st[:, 0:4, :], lstm_r[:, 0:4, :])
                nc.scalar.dma_start(bq_sb[:], bq_d.rearrange("(t p) -> p t", p=128))
                nc.scalar.dma_start(Wq_h[:], Wq_d.rearrange("(t p) a -> p t a", p=128))
                nc.sync.dma_start_transpose(XkT[:, 0], xk_st[:, 0:4, :])
                nc.sync.dma_start_transpose(XqT[:, 0], xq_st[:, 0:4, :])
                nc.sync.dma_start(xk_st[:, 4:8, :], flow_r[:, 4:8, :])
                nc.sync.dma_start(xq_st[:, 4:8, :], lstm_r[:, 4:8, :])
                nc.sync.dma_start_transpose(XkT[:, 1], xk_st[:, 4:8, :])
                nc.sync.dma_start_transpose(XqT[:, 1], xq_st[:, 4:8, :])
                nc.scalar.dma_start(bv_sb[:], bv_d.rearrange("(t p) -> p t", p=128))
                nc.scalar.dma_start(Wv_h[:], Wv_d.rearrange("(t p) a -> p t a", p=128))
                nc.sync.dma_start(xk_st[:, 8:12, :], flow_r[:, 8:12, :])
                nc.sync.dma_start_transpose(XkT[:, 2], xk_st[:, 8:12, :])
                nc.sync.dma_start(xk_st[:, 12:16, :], flow_r[:, 12:16, :])
                nc.sync.dma_start_transpose(XkT[:, 3], xk_st[:, 12:16, :])
                nc.scalar.dma_start(Wo_bf[:], Wo_d.rearrange("(t p) e -> p t e", p=128))
            else:
                # second batch: not latency-critical; keep everything on sync
                # so nothing queues in front of batch-0 compute on scalar
                nc.sync.dma_start(xq_st[:], lstm_r[:])
                nc.sync.dma_start_transpose(XqT[:], xq_st[:])
                nc.sync.dma_start(xk_st[:, 0:8, :], flow_r[:, 0:8, :])
                nc.sync.dma_start_transpose(XkT[:, 0:2], xk_st[:, 0:8, :])
                nc.sync.dma_start(xk_st[:, 8:16, :], flow_r[:, 8:16, :])
                nc.sync.dma_start_transpose(XkT[:, 2:4], xk_st[:, 8:16, :])
                bo_bcast_ap = bass.AP(tensor=bo_d.tensor, offset=bo_d.offset,
                                      ap=[[0, 128]] + list(bo_d.ap))
                nc.gpsimd.dma_start(boB[:], bo_bcast_ap)
            return XqT, XkT

        def compute_phase(b, XqT, XkT, bias_out, compute_bias):
            # B+C) projections and scores emitted in need order, so the PE
            # always has runnable work while the staging DMAs trickle in:
            # q(qc0), k(0), scores(qc0,kc0), q(qc1), k(1), scores(...), ...
            qT_c = [big.tile([128, AT, 512], f16, name=f"qTc{qc}", tag=f"qt{qc}") for qc in range(QC)]
            kT_c = [big.tile([128, AT, 512], f16, name=f"kTc{kc}", tag=f"kt{kc}") for kc in range(KC)]
            v_bf = big.tile([128, KT, A], bf16, tag="v")
            expT_c = [big.tile([128, KT, 512], bf16, name=f"expTc{qc}", tag=f"expt{qc}") for qc in range(QC)]
            dacc_c = [big.tile([128, 512], f32, name=f"daccc{qc}", tag=f"dacc{qc}") for qc in range(QC)]

            def q_proj(qc):
                for at in range(AT):
                    p = pp.tile([128, 512], f32, tag="pp")
                    for es in range(ET):
                        nc.tensor.matmul(
                            p[:], Wq_h[:, es, at * 128:(at + 1) * 128],
                            XqT[:, qc, :, es, :],
                            start=(es == 0), stop=(es == ET - 1))
                    nc.vector.tensor_scalar(
                        out=qT_c[qc][:, at, :], in0=p[:],
                        scalar1=bq_sb[:, at:at + 1], scalar2=None,
                        op0=mybir.AluOpType.add)

            def k_proj(kc):
                for at in range(AT):
                    p = pp.tile([128, 512], f32, tag="pp")
                    for fs in range(FT):
                        nc.tensor.matmul(
                            p[:], Wk_h[:, fs, at * 128:(at + 1) * 128],
                            XkT[:, kc, :, fs, :],
                            start=(fs == 0), stop=(fs == FT - 1))
                    nc.vector.tensor_scalar(
                        out=kT_c[kc][:, at, :], in0=p[:],
                        scalar1=bk_sb[:, at:at + 1], scalar2=None,
                        op0=mybir.AluOpType.add)

            def v_proj(lts):
                for lt in lts:
                    p = pp.tile([128, 512], f32, tag="pp")
                    for fs in range(FT):
                        nc.tensor.matmul(
                            p[:], XkT[:, lt // 4, lt % 4, fs, :], Wv_h[:, fs, :],
                            start=(fs == 0), stop=(fs == FT - 1))
                    nc.scalar.copy(v_bf[:, lt, :], p[:])

            def scores(qc, lts):
                for lt in lts:
                    kc, ko = lt // 4, lt % 4
                    p = pp.tile([128, 512], f32, tag="pp")
                    for at in range(AT):
                        nc.tensor.matmul(
                            p[:], kT_c[kc][:, at, ko * 128:(ko + 1) * 128],
                            qT_c[qc][:, at, :],
                            start=(at == 0), stop=(at == AT - 1))
                    nc.scalar.activation(
                        out=expT_c[qc][:, lt, :], in_=p[:],
                        func=mybir.ActivationFunctionType.Exp,
                        bias=negG[:], scale=1.0)
                    if lt == 0:
                        nc.vector.tensor_copy(dacc_c[qc][:], expT_c[qc][:, 0, :])
                    else:
                        nc.vector.tensor_add(dacc_c[qc][:], dacc_c[qc][:],
                                             expT_c[qc][:, lt, :])

            k_proj(0)
            q_proj(0)
            scores(0, range(0, 4))
            k_proj(1)
            q_proj(1)
            scores(0, range(4, 8))
            scores(1, range(0, 4))
            k_proj(2)
            v_proj(range(0, 8))
            scores(0, range(8, 12))
            scores(1, range(4, 8))
            k_proj(3)
            v_proj(range(8, 16))
            scores(0, range(12, 16))
            if compute_bias:
                # bias_out[p,e] = sum_a bv[a]*Wo[a,e] + bo[e]; emitted
                # mid-stream so a late Wo load can never stall the PE head
                ps_bo = pp.tile([128, E], f32, tag="pp")
                for at in range(AT):
                    bv_rep = small.tile([128, 128], bf16, tag="bvrep")
                    nc.vector.tensor_scalar_mul(bv_rep[:], ones128_bf[:],
                                                bv_sb[:, at:at + 1])
                    nc.tensor.matmul(ps_bo[:], bv_rep[:], Wo_bf[:, at, :],
                                     start=(at == 0), stop=(at == AT - 1))
                nc.vector.tensor_add(bias_out[:], ps_bo[:], boB[:])
            scores(1, range(8, 16))

            ps_d = pdp.tile([128, 8], f32, tag="pd")
            recipD = small.tile([128, 8], f32, tag="recip")

            # D) context (unnormalized, transposed) + final projection
            CT_c = [big.tile([128, AT, 512], bf16, name=f"CTc{qc}", tag=f"ct{qc}") for qc in range(QC)]
            for qc in range(QC):
                for at in range(AT):
                    p = pp.tile([128, 512], f32, tag="pp")
                    for lt in range(KT):
                        nc.tensor.matmul(
                            p[:], v_bf[:, lt, at * 128:(at + 1) * 128],
                            expT_c[qc][:, lt, :],
                            start=(lt == 0), stop=(lt == KT - 1))
                    nc.scalar.copy(CT_c[qc][:, at, :], p[:])

                dacc_bf = small.tile([128, 512], bf16, name=f"daccbf{qc}",
                                     tag=f"daccbf{qc}")
                nc.vector.tensor_copy(dacc_bf[:], dacc_c[qc][:])
                for qo in range(4):
                    qt = qc * 4 + qo
                    nc.tensor.matmul(ps_d[:, qt:qt + 1],
                                     dacc_bf[:, qo * 128:(qo + 1) * 128],
                                     ones128_bf[:, 0:1],
                                     start=True, stop=True)
                nc.vector.reciprocal(recipD[:, qc * 4:(qc + 1) * 4],
                                     ps_d[:, qc * 4:(qc + 1) * 4])

                for qo in range(4):
                    qt = qc * 4 + qo
                    p = pp.tile([128, 512], f32, tag="pp")
                    for at in range(AT):
                        nc.tensor.matmul(
                            p[:], CT_c[qc][:, at, qo * 128:(qo + 1) * 128],
                            Wo_bf[:, at, :],
                            start=(at == 0), stop=(at == AT - 1))
                    o_sb = small.tile([128, E], f32, tag="osb")
                    nc.scalar.activation(
                        out=o_sb[:], in_=p[:],
                        func=mybir.ActivationFunctionType.Copy,
                        scale=recipD[:, qt:qt + 1])
                    nc.vector.tensor_add(o_sb[:], o_sb[:], bias_out[:])
                    nc.scalar.dma_start(out_d[b, qt * 128:(qt + 1) * 128, :], o_sb[:])

        bias_out = wp.tile([128, E], f32)
        staged = [stage_phase(b, first=(b == 0)) for b in range(BL)]
        for b in range(BL):
            compute_phase(b, *staged[b], bias_out, compute_bias=(b == 0))


_NC_CACHE = []


def _get_nc():
    if not _NC_CACHE:
        nc = bacc.Bacc("TRN2", target_bir_lowering=False, debug=False)
        with tile.TileContext(nc) as tc:
            _body(tc)
        nc.compile()
        _NC_CACHE.append(nc)
    return _NC_CACHE[0]


def kernel(trace=False, **inputs):
    import ml_dtypes
    bf = ml_dtypes.bfloat16
    lstm = np.ascontiguousarray(
        np.asarray(inputs["lstm_embeddings"], dtype=np.float32).astype(bf))
    flow = np.ascontiguousarray(
        np.asarray(inputs["optical_flow_features"], dtype=np.float32).astype(bf))
    base = {k: np.ascontiguousarray(np.asarray(inputs[k], dtype=np.float32))
            for k in ("bq", "bk", "bv", "bo")}
    for k in ("Wq", "Wk", "Wv"):
        base[k] = np.ascontiguousarray(
            np.asarray(inputs[k], dtype=np.float32).astype(np.float16))
    base["Wo"] = np.ascontiguousarray(
        np.asarray(inputs["Wo"], dtype=np.float32).astype(bf))

    nc = _get_nc()
    in_maps = []
    for c in range(NCORES):
        m = dict(base)
        m["lstm_embeddings"] = lstm[c * BL:(c + 1) * BL]
        m["optical_flow_features"] = flow[c * BL:(c + 1) * BL]
        in_maps.append(m)

    kw = {}
    if trace:
        kw = dict(trace=True, trace_cores=[0])
    res = run_bass_kernel_spmd(nc, in_maps, core_ids=list(range(NCORES)), **kw)
    out = np.concatenate([r["out"] for r in res.results], axis=0)
    if trace:
        return out, res
    return out
